# revision 3
# baseline (speedup 1.0000x reference)
"""MoE (top-2 of 8 experts) Trainium2 kernel.

Strategy: expert-quad parallel with F-dim sharding. The 8 experts are
split into 2 quads of 4 experts; each quad is served by 4 NeuronCores,
each core computing a 768-wide slice of the F=3072 hidden dim for ALL
4 experts of its quad over the quad's full routed-token stream. This
balances per-core matmul work to ~the quad mean (vs the max expert
load under 1-expert-per-core), because every core of a quad does
exactly the same token columns. Host computes the (tiny) router +
top-2 dispatch, sums the 4 partial outputs per quad, applies the
renormalized gates + b2, and scatter-adds into the full output.

Shapes (hardcoded from the problem spec): B=4, S=2048, D=768, E=8,
F=4*D=3072, TOP_K=2. Per-core F slice FS=768 (6 tiles of 128).
"""

import os
import sys
import types
from itertools import combinations

import numpy as np
import ml_dtypes

# concourse.bass_utils imports antenv.axon_hooks when tracing is requested
# (e.g. BASS_TRACE=1); some deployments lack that module. Provide a stub so
# tracing degrades gracefully (run without trace) instead of crashing.
try:
    from antenv import axon_hooks as _axon_hooks  # noqa: F401
except ImportError:
    _m = types.ModuleType("antenv.axon_hooks")
    _m._hook = None
    _m.set_axon_ntff_profile_hook = lambda h: setattr(_m, "_hook", h)
    _m.get_axon_ntff_profile_hook = lambda: _m._hook
    sys.modules["antenv.axon_hooks"] = _m
    try:
        import antenv

        antenv.axon_hooks = _m
    except ImportError:
        pass

import concourse.bass as bass
import concourse.tile as tile
from concourse import bacc, mybir
from concourse.bass_utils import run_bass_kernel_spmd

P = 128
D = 768
F = 3072
E = 8
TOP_K = 2
N_CORES = 8
NSLOT = 4       # experts per quad (= cores per quad)
FS = F // NSLOT # per-core F slice
nD = D // P     # 6
nFs = FS // P   # 6

bf16 = mybir.dt.bfloat16
f32 = mybir.dt.float32

# Stash of the most recent BassKernelResults (for test harness introspection).
last_results = None


def _chunks_of(total, size):
    """Split into chunks of `size`, avoiding a tail chunk under 256 when
    possible (small-N matmuls pay proportionally more issue overhead)."""
    out = []
    t0 = 0
    while t0 < total:
        rem = total - t0
        if size < rem < size + 256 and rem - 256 >= 256:
            out.append((t0, rem - 256))
            out.append((t0 + rem - 256, 256))
            break
        cs = min(size, rem)
        out.append((t0, cs))
        t0 += cs
    return out


def _build(slot_sizes, slot_order):
    """Quad-FFN kernel: for 4 expert slots, yT[:, cols(slot)] +=
    (gelu(x[cols] @ w1s[slot] + b1s[slot]) @ w2s[slot]).T  over this
    core's F slice. Partial outputs (summed across the quad's 4 cores
    on the host). All contraction dims land on SBUF partitions via
    host-side pre-permutation; no on-device transpose.

    Inputs:
      xT  [D, TOT]            x (pair-stream) transposed, bf16
      w1q [NSLOT, nFs, P, nD*P]  w1q[s, fi, p, d*P+c] = w1slice[d*P+p, fi*P+c]
      w2q [NSLOT, nFs, P, D]     w2q[s, f, p, d] = w2slice[f*P+p, d]
      b1q [P, NSLOT, nFs]        b1q[p, s, fi] = b1slice[fi*P+p]
    Output yT [D, TOT] bf16 (host sums 4 cores, transposes back).
    slot_sizes: padded column count per slot; slot_order: processing
    order of slots (chosen so the final chunk is the smallest).
    """
    offs = np.cumsum([0] + [slot_sizes[s] for s in slot_order])
    TOT = int(offs[-1])
    # flattened chunk schedule: (slot, col0, cs)
    sched = []
    for k, s in enumerate(slot_order):
        for t0, cs in _chunks_of(slot_sizes[s], 512):
            sched.append((s, int(offs[k]) + t0, cs))

    nc = bacc.Bacc(
        "TRN2", target_bir_lowering=False, debug=False, num_devices=N_CORES
    )
    xT = nc.declare_dram_parameter("xT", [D, TOT], bf16, isOutput=False)
    w1q = nc.declare_dram_parameter("w1q", [NSLOT, nFs, P, nD * P], bf16, isOutput=False)
    w2q = nc.declare_dram_parameter("w2q", [NSLOT, nFs, P, D], bf16, isOutput=False)
    b1q = nc.declare_dram_parameter("b1q", [P, NSLOT, nFs], f32, isOutput=False)
    yT = nc.declare_dram_parameter("yT", [D, TOT], bf16, isOutput=True)

    XPRE = 8  # x chunks prefetched ahead of compute

    with tile.TileContext(nc) as tc:
        with (
            tc.tile_pool(name="const", bufs=1) as const_pool,
            tc.tile_pool(name="xpool", bufs=XPRE + 2) as xpool,
            tc.tile_pool(name="hpool", bufs=2) as hpool,
            tc.tile_pool(name="psum1", bufs=4, space="PSUM") as psum1,
            tc.tile_pool(name="psum2", bufs=3, space="PSUM") as psum2,
            tc.tile_pool(name="outp", bufs=3) as outp,
        ):
            xT_r = xT.rearrange("(o p) t -> p o t", p=P)
            yT_r = yT.rearrange("(o p) t -> p o t", p=P)

            # b1 first (tiny; the gelu activations carry it as a pointer
            # operand with only one sync-wait slot, so pre-touch it on the
            # scalar engine right away).
            b1_sb = const_pool.tile([P, NSLOT, nFs], f32)
            nc.sync.dma_start(b1_sb[:], b1q[:, :, :])
            scratch = const_pool.tile([P, 1], f32)
            nc.scalar.copy(scratch[:], b1_sb[:, 0, 0:1])

            # Input DMAs interleaved so chunk-0 compute gates on only
            # ~1.3 MB (x0 + first half of slot0 w1) while later weights
            # stream behind the first chunks' compute.
            x_sb = {}

            def issue_x(ci):
                s, t0, cs = sched[ci]
                xt = xpool.tile([P, nD, 512], bf16, tag="x")
                nc.sync.dma_start(xt[:, :, :cs], xT_r[:, :, t0 : t0 + cs])
                x_sb[ci] = xt

            w1_sb = {}
            w2_sb = {}

            def issue_w1(s, half):
                t = const_pool.tile([P, nFs // 2, nD * P], bf16, tag=f"w1_{s}_{half}")
                src = w1q[s, half * (nFs // 2) : (half + 1) * (nFs // 2)]
                nc.sync.dma_start(t[:], src.rearrange("f p dc -> p f dc"))
                w1_sb[(s, half)] = t

            def issue_w2(s):
                t = const_pool.tile([P, nFs, D], bf16, tag=f"w2_{s}")
                nc.sync.dma_start(t[:], w2q[s].rearrange("f p d -> p f d"))
                w2_sb[s] = t

            issue_x(0)
            issue_w1(slot_order[0], 0)
            issue_w1(slot_order[0], 1)
            issue_x(1)
            issue_w2(slot_order[0])
            issue_x(2)
            issue_w1(slot_order[1], 0)
            issue_w1(slot_order[1], 1)
            issue_x(3)
            issue_w2(slot_order[1])
            issue_x(4)
            issue_w1(slot_order[2], 0)
            issue_w1(slot_order[2], 1)
            issue_x(5)
            issue_w2(slot_order[2])
            issue_x(6)
            issue_w1(slot_order[3], 0)
            issue_w1(slot_order[3], 1)
            issue_x(7)
            issue_w2(slot_order[3])

            def w1_tile(s, fi, d):
                return w1_sb[(s, fi // 3)][:, fi % 3, d * P : (d + 1) * P]

            def w2_tile(s, fi, do):
                return w2_sb[s][:, fi, do * P : (do + 1) * P]

            # Dummy matmuls on a zeroed tile: keeps the PE active through
            # the HAM ramp (~3.3 us at half clock) while the first input
            # DMAs stream in, so real matmuls start at 2.4 GHz.
            warm_src = const_pool.tile([P, P], bf16)
            nc.any.memset(warm_src[:], 0.0)
            for _w in range(17):
                pw = psum1.tile([P, 512], f32, tag="ph", name="pw")
                for k in range(4):
                    nc.tensor.matmul(
                        pw[:, :64],
                        lhsT=warm_src[:],
                        rhs=warm_src[:, :64],
                        start=(k == 0),
                        stop=(k == 3),
                    )

            h_of = {}

            def g1(ci):
                # h[f, tok] = gelu(sum_d w1[d, f] * x[d, tok] + b1[f])
                s, t0, cs = sched[ci]
                h = hpool.tile([P, nFs, 512], bf16, tag="h")
                for fi in range(nFs):
                    ph = psum1.tile([P, 512], f32, tag="ph")
                    for d in range(nD):
                        nc.tensor.matmul(
                            ph[:, :cs],
                            lhsT=w1_tile(s, fi, d),
                            rhs=x_sb[ci][:, d, :cs],
                            start=(d == 0),
                            stop=(d == nD - 1),
                        )
                    nc.scalar.activation(
                        h[:, fi, :cs],
                        ph[:, :cs],
                        mybir.ActivationFunctionType.Gelu,
                        bias=b1_sb[:, s, fi : fi + 1],
                    )
                h_of[ci] = h

            def g2(ci):
                # yT[dout, tok] = sum_f w2[f, dout] * h[f, tok]
                # do-major: each dout's psum completes early so its
                # copy-back overlaps the next dout's matmuls.
                s, t0, cs = sched[ci]
                h = h_of.pop(ci)
                ot = outp.tile([P, nD, 512], bf16, tag="ot")
                for do in range(nD):
                    py = psum2.tile([P, 512], f32, tag="py")
                    for fi in range(nFs):
                        nc.tensor.matmul(
                            py[:, :cs],
                            lhsT=w2_tile(s, fi, do),
                            rhs=h[:, fi, :cs],
                            start=(fi == 0),
                            stop=(fi == nFs - 1),
                        )
                    nc.vector.tensor_copy(ot[:, do, :cs], py[:, :cs])
                nc.sync.dma_start(yT_r[:, :, t0 : t0 + cs], ot[:, :, :cs])

            # Software pipeline: g1(c+1) issues between g1(c) and g2(c)
            # so every gelu has a full chunk of PE time to land before
            # its h is consumed, and the PE never waits on the scalar
            # engine at chunk boundaries.
            nch = len(sched)
            g1(0)
            for ci in range(1, nch):
                if ci + XPRE - 1 < nch:
                    issue_x(ci + XPRE - 1)
                g1(ci)
                g2(ci - 1)
            g2(nch - 1)
    nc.compile()
    return nc, TOT, offs


def _route(xf, router_w, router_b):
    """Top-2 routing, numpy fp32. Returns (idx1, idx2, g1, g2)."""
    logits = xf @ router_w + router_b
    m = logits.max(axis=-1, keepdims=True)
    p = np.exp(logits - m, dtype=np.float32)
    p /= p.sum(axis=-1, keepdims=True)
    # top-2 indices, ties -> lower index first (matches jax.lax.top_k)
    part = np.argpartition(-p, 1, axis=-1)[:, :2]
    pv = np.take_along_axis(p, part, axis=-1)
    swap = (pv[:, 1] > pv[:, 0]) | ((pv[:, 1] == pv[:, 0]) & (part[:, 1] < part[:, 0]))
    i1 = np.where(swap, part[:, 1], part[:, 0])
    i2 = np.where(swap, part[:, 0], part[:, 1])
    p1 = np.take_along_axis(p, i1[:, None], axis=-1)[:, 0]
    p2 = np.take_along_axis(p, i2[:, None], axis=-1)[:, 0]
    s = p1 + p2
    return i1, i2, p1 / s, p2 / s


def _ceil8(n):
    return -(-n // 8) * 8


def kernel(x, router_w, router_b, w1, b1, w2, b2):
    global last_results
    x = np.asarray(x, dtype=np.float32)
    router_w = np.asarray(router_w, dtype=np.float32)
    router_b = np.asarray(router_b, dtype=np.float32)
    w1 = np.asarray(w1, dtype=np.float32)
    b1 = np.asarray(b1, dtype=np.float32)
    w2 = np.asarray(w2, dtype=np.float32)
    b2 = np.asarray(b2, dtype=np.float32)

    B, S, _ = x.shape
    T = B * S
    xf = x.reshape(T, D)

    i1, i2, g1_, g2_ = _route(xf, router_w, router_b)

    tok_lists = []
    gate_lists = []
    for e in range(E):
        m1 = i1 == e
        m2 = i2 == e
        toks = np.nonzero(m1 | m2)[0]
        gates = np.where(m1[toks], g1_[toks], g2_[toks]).astype(np.float32)
        tok_lists.append(toks)
        gate_lists.append(gates)
    counts = np.array([len(t) for t in tok_lists])

    # Split experts into 2 quads of 4 minimizing the shared (slot-wise max)
    # schedule size. Each quad's experts sorted desc onto slots.
    best = None
    for combo in combinations(range(E), NSLOT):
        if 0 not in combo:
            continue
        qa = sorted(combo, key=lambda e: -counts[e])
        qb = sorted(set(range(E)) - set(combo), key=lambda e: -counts[e])
        sizes = [
            _ceil8(max(counts[qa[k]], counts[qb[k]])) for k in range(NSLOT)
        ]
        tot = sum(sizes)
        if best is None or tot < best[0]:
            best = (tot, qa, qb, sizes)
    _, quad_a, quad_b, slot_sizes = best
    quads = [quad_a, quad_b]

    # Process slots so the final chunk of the kernel is the smallest one.
    last_cs = [_chunks_of(slot_sizes[s], 512)[-1][1] for s in range(NSLOT)]
    slot_order = sorted(range(NSLOT), key=lambda s: -last_cs[s])

    nc, TOT, offs = _build(slot_sizes, slot_order)
    slot_off = {s: int(offs[k]) for k, s in enumerate(slot_order)}

    xf_b = xf.astype(ml_dtypes.bfloat16)
    w1_b = w1.astype(ml_dtypes.bfloat16)
    w2_b = w2.astype(ml_dtypes.bfloat16)

    in_maps = []
    for q in range(2):
        xT = np.zeros((D, TOT), dtype=ml_dtypes.bfloat16)
        for s in range(NSLOT):
            e = quads[q][s]
            toks = tok_lists[e]
            off = slot_off[s]
            xT[:, off : off + len(toks)] = xf_b[toks].T
        for fs in range(NSLOT):
            lo = fs * FS
            w1q = np.empty((NSLOT, nFs, P, nD * P), dtype=ml_dtypes.bfloat16)
            w2q = np.empty((NSLOT, nFs, P, D), dtype=ml_dtypes.bfloat16)
            b1q = np.empty((P, NSLOT, nFs), dtype=np.float32)
            for s in range(NSLOT):
                e = quads[q][s]
                # w1 slice [D, FS] -> [nFs, P, nD*P]
                w1s = w1_b[e][:, lo : lo + FS]
                w1q[s] = (
                    w1s.reshape(nD, P, nFs, P).transpose(2, 1, 0, 3).reshape(nFs, P, nD * P)
                )
                # w2 slice [FS, D] -> [nFs, P, D]
                w2q[s] = w2_b[e][lo : lo + FS].reshape(nFs, P, D)
                b1q[:, s, :] = b1[e][lo : lo + FS].reshape(nFs, P).T
            in_maps.append({"xT": xT, "w1q": w1q, "w2q": w2q, "b1q": b1q})

    trace = bool(int(os.environ.get("KERNEL_TRACE", "0")))
    last_results = run_bass_kernel_spmd(
        nc, in_maps, core_ids=list(range(N_CORES)), trace=trace
    )

    out = np.zeros((T, D), dtype=np.float32)
    for q in range(2):
        ysum = np.zeros((D, TOT), dtype=np.float32)
        for fs in range(NSLOT):
            ysum += last_results.results[q * NSLOT + fs]["yT"].astype(np.float32)
        for s in range(NSLOT):
            e = quads[q][s]
            toks = tok_lists[e]
            off = slot_off[s]
            ye = ysum[:, off : off + len(toks)].T
            out[toks] += gate_lists[e][:, None] * (ye + b2[e][None, :])
    return out.reshape(B, S, D)


# revision 4
# speedup vs baseline: 1.0146x; 1.0146x over previous
"""MoE (top-2 of 8 experts) Trainium2 kernel.

Strategy: expert-quad parallel with F-dim sharding. The 8 experts are
split into 2 quads of 4 experts; each quad is served by 4 NeuronCores,
each core computing a 768-wide slice of the F=3072 hidden dim for ALL
4 experts of its quad over the quad's full routed-token stream. This
balances per-core matmul work to ~the quad mean (vs the max expert
load under 1-expert-per-core), because every core of a quad does
exactly the same token columns. Host computes the (tiny) router +
top-2 dispatch, sums the 4 partial outputs per quad, applies the
renormalized gates + b2, and scatter-adds into the full output.

All DRAM parameters are pre-tiled on the host into the exact SBUF
layout (partition-major, contiguous 6-9 KB per partition row) so each
DMA is descriptor-cheap; strided/rearranging DMAs run at only
~100 GB/s aggregate (descriptor-bound) and starve the PE.

Shapes (hardcoded from the problem spec): B=4, S=2048, D=768, E=8,
F=4*D=3072, TOP_K=2. Per-core F slice FS=768 (6 tiles of 128).
"""

import os
import sys
import types
from itertools import combinations

import numpy as np
import ml_dtypes

# concourse.bass_utils imports antenv.axon_hooks when tracing is requested
# (e.g. BASS_TRACE=1); some deployments lack that module. Provide a stub so
# tracing degrades gracefully (run without trace) instead of crashing.
try:
    from antenv import axon_hooks as _axon_hooks  # noqa: F401
except ImportError:
    _m = types.ModuleType("antenv.axon_hooks")
    _m._hook = None
    _m.set_axon_ntff_profile_hook = lambda h: setattr(_m, "_hook", h)
    _m.get_axon_ntff_profile_hook = lambda: _m._hook
    sys.modules["antenv.axon_hooks"] = _m
    try:
        import antenv

        antenv.axon_hooks = _m
    except ImportError:
        pass

import concourse.bass as bass
import concourse.tile as tile
from concourse import bacc, mybir
from concourse.bass_utils import run_bass_kernel_spmd

P = 128
D = 768
F = 3072
E = 8
TOP_K = 2
N_CORES = 8
NSLOT = 4        # experts per quad (= cores per quad)
FS = F // NSLOT  # per-core F slice
nD = D // P      # 6
nFs = FS // P    # 6
CHK = 512        # token chunk

bf16 = mybir.dt.bfloat16
f32 = mybir.dt.float32

# Stash of the most recent BassKernelResults (for test harness introspection).
last_results = None


def _chunks_of(total, size):
    """Split into chunks of `size`, avoiding a tail chunk under 256 when
    possible (small-N matmuls pay proportionally more issue overhead)."""
    out = []
    t0 = 0
    while t0 < total:
        rem = total - t0
        if size < rem < size + 256 and rem - 256 >= 256:
            out.append((t0, rem - 256))
            out.append((t0 + rem - 256, 256))
            break
        cs = min(size, rem)
        out.append((t0, cs))
        t0 += cs
    return out


def _build(sched):
    """Quad-FFN kernel over a static chunk schedule.

    For each chunk ci with slot s and width cs:
      h = gelu(x[ci] @ w1[s] + b1[s])   (this core's F slice, 768 wide)
      y[ci] = h @ w2[s]                  (partial over the F slice)

    All DRAM tensors are pre-tiled host-side to the SBUF layout:
      xh  [NCH, P, nD*CHK]      xh[c, p, d*CHK + t] = x[tok(c,t), d*P + p]
      w1h [NSLOT, 2, P, 3*nD*P] w1h[s, g, p, (f*nD + d)*P + c] =
                                  w1slice[d*P + p, (3g + f)*P + c]
      w2h [NSLOT, P, nFs*D]     w2h[s, p, f*D + d] = w2slice[f*P + p, d]
      b1h [P, NSLOT*nFs]        b1h[p, s*nFs + f] = b1slice_s[f*P + p]
    Output yh [NCH, P, nD*CHK]  yh[c, p, o*CHK + t] = y[tok(c,t), o*P + p]
    (host sums the quad's 4 cores and un-permutes; both free).
    """
    NCH = len(sched)
    nc = bacc.Bacc(
        "TRN2", target_bir_lowering=False, debug=False, num_devices=N_CORES
    )
    xh = nc.declare_dram_parameter("xh", [NCH, P, nD * CHK], bf16, isOutput=False)
    w1h = nc.declare_dram_parameter("w1h", [NSLOT, 2, P, 3 * nD * P], bf16, isOutput=False)
    w2h = nc.declare_dram_parameter("w2h", [NSLOT, P, nFs * D], bf16, isOutput=False)
    b1h = nc.declare_dram_parameter("b1h", [P, NSLOT * nFs], f32, isOutput=False)
    yh = nc.declare_dram_parameter("yh", [NCH, P, nD * CHK], bf16, isOutput=True)

    XPRE = 8  # x chunks prefetched ahead of compute

    with tile.TileContext(nc) as tc:
        with (
            tc.tile_pool(name="const", bufs=1) as const_pool,
            tc.tile_pool(name="xpool", bufs=XPRE + 2) as xpool,
            tc.tile_pool(name="hpool", bufs=2) as hpool,
            tc.tile_pool(name="psum1", bufs=4, space="PSUM") as psum1,
            tc.tile_pool(name="psum2", bufs=3, space="PSUM") as psum2,
            tc.tile_pool(name="outp", bufs=3) as outp,
        ):
            # b1 first (tiny; the gelu activations carry it as a pointer
            # operand with only one sync-wait slot, so pre-touch it on the
            # scalar engine right away).
            b1_sb = const_pool.tile([P, NSLOT * nFs], f32)
            nc.sync.dma_start(b1_sb[:], b1h[:, :])
            scratch = const_pool.tile([P, 1], f32)
            nc.scalar.copy(scratch[:], b1_sb[:, 0:1])

            # Input DMAs interleaved so chunk-0 compute gates on only
            # ~1.3 MB (x0 + first half of slot0 w1) while later weights
            # stream behind the first chunks' compute.
            x_sb = {}

            def issue_x(ci):
                xt = xpool.tile([P, nD, CHK], bf16, tag="x")
                nc.sync.dma_start(xt[:], xh[ci].rearrange("p (d t) -> p d t", d=nD))
                x_sb[ci] = xt

            w1_sb = {}
            w2_sb = {}

            def issue_w1(s, half):
                t = const_pool.tile([P, 3, nD * P], bf16, tag=f"w1_{s}_{half}")
                nc.sync.dma_start(
                    t[:], w1h[s, half].rearrange("p (f dc) -> p f dc", f=3)
                )
                w1_sb[(s, half)] = t

            def issue_w2(s):
                t = const_pool.tile([P, nFs, D], bf16, tag=f"w2_{s}")
                nc.sync.dma_start(t[:], w2h[s].rearrange("p (f d) -> p f d", f=nFs))
                w2_sb[s] = t

            slot_first = []
            for s, _ in sched:
                if s not in slot_first:
                    slot_first.append(s)
            issue_x(0)
            issue_w1(slot_first[0], 0)
            issue_w1(slot_first[0], 1)
            issue_x(1)
            issue_w2(slot_first[0])
            issue_x(2)
            issue_w1(slot_first[1], 0)
            issue_w1(slot_first[1], 1)
            issue_x(3)
            issue_w2(slot_first[1])
            issue_x(4)
            issue_w1(slot_first[2], 0)
            issue_w1(slot_first[2], 1)
            issue_x(5)
            issue_w2(slot_first[2])
            issue_x(6)
            issue_w1(slot_first[3], 0)
            issue_w1(slot_first[3], 1)
            issue_x(7)
            issue_w2(slot_first[3])

            def w1_tile(s, fi, d):
                return w1_sb[(s, fi // 3)][:, fi % 3, d * P : (d + 1) * P]

            def w2_tile(s, fi, do):
                return w2_sb[s][:, fi, do * P : (do + 1) * P]

            # Dummy matmuls on a zeroed tile: keeps the PE active through
            # the HAM ramp (~3.3 us at half clock) while the first input
            # DMAs stream in, so real matmuls start at 2.4 GHz.
            warm_src = const_pool.tile([P, P], bf16)
            nc.any.memset(warm_src[:], 0.0)
            for _w in range(17):
                pw = psum1.tile([P, CHK], f32, tag="ph", name="pw")
                for k in range(4):
                    nc.tensor.matmul(
                        pw[:, :64],
                        lhsT=warm_src[:],
                        rhs=warm_src[:, :64],
                        start=(k == 0),
                        stop=(k == 3),
                    )

            h_of = {}

            def g1(ci):
                # h[f, tok] = gelu(sum_d w1[d, f] * x[d, tok] + b1[f])
                s, cs = sched[ci]
                h = hpool.tile([P, nFs, CHK], bf16, tag="h")
                for fi in range(nFs):
                    ph = psum1.tile([P, CHK], f32, tag="ph")
                    for d in range(nD):
                        nc.tensor.matmul(
                            ph[:, :cs],
                            lhsT=w1_tile(s, fi, d),
                            rhs=x_sb[ci][:, d, :cs],
                            start=(d == 0),
                            stop=(d == nD - 1),
                        )
                    nc.scalar.activation(
                        h[:, fi, :cs],
                        ph[:, :cs],
                        mybir.ActivationFunctionType.Gelu,
                        bias=b1_sb[:, s * nFs + fi : s * nFs + fi + 1],
                    )
                h_of[ci] = h

            def g2(ci):
                # y[dout, tok] = sum_f w2[f, dout] * h[f, tok]
                # do-major: each dout's psum completes early so its
                # copy-back overlaps the next dout's matmuls.
                s, cs = sched[ci]
                h = h_of.pop(ci)
                ot = outp.tile([P, nD, CHK], bf16, tag="ot")
                for do in range(nD):
                    py = psum2.tile([P, CHK], f32, tag="py")
                    for fi in range(nFs):
                        nc.tensor.matmul(
                            py[:, :cs],
                            lhsT=w2_tile(s, fi, do),
                            rhs=h[:, fi, :cs],
                            start=(fi == 0),
                            stop=(fi == nFs - 1),
                        )
                    nc.vector.tensor_copy(ot[:, do, :cs], py[:, :cs])
                nc.sync.dma_start(
                    yh[ci].rearrange("p (d t) -> p d t", d=nD), ot[:]
                )

            # Software pipeline: g1(c+1) issues between g1(c) and g2(c)
            # so every gelu has a full chunk of PE time to land before
            # its h is consumed, and the PE never waits on the scalar
            # engine at chunk boundaries.
            g1(0)
            for ci in range(1, NCH):
                if ci + XPRE - 1 < NCH:
                    issue_x(ci + XPRE - 1)
                g1(ci)
                g2(ci - 1)
            g2(NCH - 1)
    nc.compile()
    return nc


def _route(xf, router_w, router_b):
    """Top-2 routing, numpy fp32. Returns (idx1, idx2, g1, g2)."""
    logits = xf @ router_w + router_b
    m = logits.max(axis=-1, keepdims=True)
    p = np.exp(logits - m, dtype=np.float32)
    p /= p.sum(axis=-1, keepdims=True)
    # top-2 indices, ties -> lower index first (matches jax.lax.top_k)
    part = np.argpartition(-p, 1, axis=-1)[:, :2]
    pv = np.take_along_axis(p, part, axis=-1)
    swap = (pv[:, 1] > pv[:, 0]) | ((pv[:, 1] == pv[:, 0]) & (part[:, 1] < part[:, 0]))
    i1 = np.where(swap, part[:, 1], part[:, 0])
    i2 = np.where(swap, part[:, 0], part[:, 1])
    p1 = np.take_along_axis(p, i1[:, None], axis=-1)[:, 0]
    p2 = np.take_along_axis(p, i2[:, None], axis=-1)[:, 0]
    s = p1 + p2
    return i1, i2, p1 / s, p2 / s


def _ceil8(n):
    return -(-n // 8) * 8


def kernel(x, router_w, router_b, w1, b1, w2, b2):
    global last_results
    x = np.asarray(x, dtype=np.float32)
    router_w = np.asarray(router_w, dtype=np.float32)
    router_b = np.asarray(router_b, dtype=np.float32)
    w1 = np.asarray(w1, dtype=np.float32)
    b1 = np.asarray(b1, dtype=np.float32)
    w2 = np.asarray(w2, dtype=np.float32)
    b2 = np.asarray(b2, dtype=np.float32)

    B, S, _ = x.shape
    T = B * S
    xf = x.reshape(T, D)

    i1, i2, g1_, g2_ = _route(xf, router_w, router_b)

    tok_lists = []
    gate_lists = []
    for e in range(E):
        m1 = i1 == e
        m2 = i2 == e
        toks = np.nonzero(m1 | m2)[0]
        gates = np.where(m1[toks], g1_[toks], g2_[toks]).astype(np.float32)
        tok_lists.append(toks)
        gate_lists.append(gates)
    counts = np.array([len(t) for t in tok_lists])

    # Split experts into 2 quads of 4 minimizing the shared (slot-wise max)
    # schedule size. Each quad's experts sorted desc onto slots.
    best = None
    for combo in combinations(range(E), NSLOT):
        if 0 not in combo:
            continue
        qa = sorted(combo, key=lambda e: -counts[e])
        qb = sorted(set(range(E)) - set(combo), key=lambda e: -counts[e])
        sizes = [
            _ceil8(max(counts[qa[k]], counts[qb[k]])) for k in range(NSLOT)
        ]
        tot = sum(sizes)
        if best is None or tot < best[0]:
            best = (tot, qa, qb, sizes)
    _, quad_a, quad_b, slot_sizes = best
    quads = [quad_a, quad_b]

    # Process slots so the final chunk of the kernel is the smallest one.
    slot_chunks = [_chunks_of(slot_sizes[s], CHK) for s in range(NSLOT)]
    slot_order = sorted(range(NSLOT), key=lambda s: -slot_chunks[s][-1][1])
    # Flattened schedule: list of (slot, cs) plus per-chunk column ranges.
    sched = []
    chunk_cols = []  # (slot, t0_in_slot, cs)
    for s in slot_order:
        for t0, cs in slot_chunks[s]:
            sched.append((s, cs))
            chunk_cols.append((s, t0, cs))
    NCH = len(sched)

    nc = _build(sched)

    xf_b = xf.astype(ml_dtypes.bfloat16)
    w1_b = w1.astype(ml_dtypes.bfloat16)
    w2_b = w2.astype(ml_dtypes.bfloat16)

    in_maps = []
    for q in range(2):
        # Per-slot xT [D, size] then cut into pre-tiled chunks.
        xh = np.zeros((NCH, P, nD * CHK), dtype=ml_dtypes.bfloat16)
        xslot = {}
        for s in range(NSLOT):
            e = quads[q][s]
            toks = tok_lists[e]
            xs = np.zeros((D, slot_sizes[s]), dtype=ml_dtypes.bfloat16)
            xs[:, : len(toks)] = xf_b[toks].T
            xslot[s] = xs
        for ci, (s, t0, cs) in enumerate(chunk_cols):
            blk = np.zeros((nD, P, CHK), dtype=ml_dtypes.bfloat16)
            blk[:, :, :cs] = xslot[s][:, t0 : t0 + cs].reshape(nD, P, cs)
            xh[ci] = blk.transpose(1, 0, 2).reshape(P, nD * CHK)
        for fs in range(NSLOT):
            lo = fs * FS
            w1h = np.empty((NSLOT, 2, P, 3 * nD * P), dtype=ml_dtypes.bfloat16)
            w2h = np.empty((NSLOT, P, nFs * D), dtype=ml_dtypes.bfloat16)
            b1h = np.empty((P, NSLOT * nFs), dtype=np.float32)
            for s in range(NSLOT):
                e = quads[q][s]
                # w1 slice [D, FS]: w1h[s, g, p, (f*nD + d)*P + c]
                #   = w1slice[d*P + p, (3g + f)*P + c]
                w1s = w1_b[e][:, lo : lo + FS]
                t = w1s.reshape(nD, P, 2, 3, P).transpose(2, 1, 3, 0, 4)
                w1h[s] = t.reshape(2, P, 3 * nD * P)
                # w2 slice [FS, D]: w2h[s, p, f*D + d] = w2slice[f*P+p, d]
                w2s = w2_b[e][lo : lo + FS]
                w2h[s] = w2s.reshape(nFs, P, D).transpose(1, 0, 2).reshape(P, nFs * D)
                b1h[:, s * nFs : (s + 1) * nFs] = (
                    b1[e][lo : lo + FS].reshape(nFs, P).T
                )
            in_maps.append({"xh": xh, "w1h": w1h, "w2h": w2h, "b1h": b1h})

    trace = bool(int(os.environ.get("KERNEL_TRACE", "0")))
    last_results = run_bass_kernel_spmd(
        nc, in_maps, core_ids=list(range(N_CORES)), trace=trace
    )

    out = np.zeros((T, D), dtype=np.float32)
    for q in range(2):
        ysum = np.zeros((NCH, P, nD * CHK), dtype=np.float32)
        for fs in range(NSLOT):
            ysum += last_results.results[q * NSLOT + fs]["yh"].astype(np.float32)
        # yh[c, p, o*CHK + t] -> per-chunk [D, cs] blocks
        ysum = ysum.reshape(NCH, P, nD, CHK)
        yslot = {s: np.empty((D, slot_sizes[s]), dtype=np.float32) for s in range(NSLOT)}
        for ci, (s, t0, cs) in enumerate(chunk_cols):
            yslot[s][:, t0 : t0 + cs] = (
                ysum[ci, :, :, :cs].transpose(1, 0, 2).reshape(D, cs)
            )
        for s in range(NSLOT):
            e = quads[q][s]
            toks = tok_lists[e]
            ye = yslot[s][:, : len(toks)].T
            out[toks] += gate_lists[e][:, None] * (ye + b2[e][None, :])
    return out.reshape(B, S, D)


# revision 9
# speedup vs baseline: 1.0180x; 1.0034x over previous
"""MoE (top-2 of 8 experts) Trainium2 kernel.

Strategy: expert-quad parallel with F-dim sharding. The 8 experts are
split into 2 quads of 4 experts; each quad is served by 4 NeuronCores,
each core computing a 768-wide slice of the F=3072 hidden dim for ALL
4 experts of its quad over the quad's full routed-token stream. This
balances per-core matmul work to ~the quad mean (vs the max expert
load under 1-expert-per-core), because every core of a quad does
exactly the same token columns. Host computes the (tiny) router +
top-2 dispatch, sums the 4 partial outputs per quad, applies the
renormalized gates + b2, and scatter-adds into the full output.

All DRAM parameters are pre-tiled on the host into the exact SBUF
layout (partition-major, contiguous 6-9 KB per partition row) so each
DMA is descriptor-cheap; strided/rearranging DMAs run at only
~100 GB/s aggregate (descriptor-bound) and starve the PE.

Shapes (hardcoded from the problem spec): B=4, S=2048, D=768, E=8,
F=4*D=3072, TOP_K=2. Per-core F slice FS=768 (6 tiles of 128).
"""

import os
import sys
import types
from itertools import combinations

import numpy as np
import ml_dtypes

# concourse.bass_utils imports antenv.axon_hooks when tracing is requested
# (e.g. BASS_TRACE=1); some deployments lack that module. Provide a stub so
# tracing degrades gracefully (run without trace) instead of crashing.
try:
    from antenv import axon_hooks as _axon_hooks  # noqa: F401
except ImportError:
    _m = types.ModuleType("antenv.axon_hooks")
    _m._hook = None
    _m.set_axon_ntff_profile_hook = lambda h: setattr(_m, "_hook", h)
    _m.get_axon_ntff_profile_hook = lambda: _m._hook
    sys.modules["antenv.axon_hooks"] = _m
    try:
        import antenv

        antenv.axon_hooks = _m
    except ImportError:
        pass

import concourse.bass as bass
import concourse.tile as tile
from concourse import bacc, mybir
from concourse.bass_utils import run_bass_kernel_spmd

P = 128
D = 768
F = 3072
E = 8
TOP_K = 2
N_CORES = 8
NSLOT = 4        # experts per quad (= cores per quad)
FS = F // NSLOT  # per-core F slice
nD = D // P      # 6
nFs = FS // P    # 6
CHK = 512        # token chunk

bf16 = mybir.dt.bfloat16
f32 = mybir.dt.float32

# Stash of the most recent BassKernelResults (for test harness introspection).
last_results = None


def _chunks_of(total, size):
    """Split into chunks of `size`, avoiding a tail chunk under 256 when
    possible (small-N matmuls pay proportionally more issue overhead)."""
    out = []
    t0 = 0
    while t0 < total:
        rem = total - t0
        if size < rem < size + 256 and rem - 256 >= 256:
            out.append((t0, rem - 256))
            out.append((t0 + rem - 256, 256))
            break
        cs = min(size, rem)
        out.append((t0, cs))
        t0 += cs
    return out


def _build(sched):
    """Quad-FFN kernel over a static chunk schedule.

    For each chunk ci with slot s and width cs:
      h = gelu(x[ci] @ w1[s] + b1[s])   (this core's F slice, 768 wide)
      y[ci] = h @ w2[s]                  (partial over the F slice)

    All DRAM tensors are pre-tiled host-side to the SBUF layout:
      xh  [NCH, P, nD*CHK]      xh[c, p, d*CHK + t] = x[tok(c,t), d*P + p]
      w1h [NSLOT, 2, P, 3*nD*P] w1h[s, g, p, (f*nD + d)*P + c] =
                                  w1slice[d*P + p, (3g + f)*P + c]
      w2h [NSLOT, P, nFs*D]     w2h[s, p, f*D + d] = w2slice[f*P + p, d]
      b1h [P, NSLOT*nFs]        b1h[p, s*nFs + f] = b1slice_s[f*P + p]
    Output yh [NCH, P, nD*CHK]  yh[c, p, o*CHK + t] = y[tok(c,t), o*P + p]
    (host sums the quad's 4 cores and un-permutes; both free).
    """
    NCH = len(sched)
    nc = bacc.Bacc(
        "TRN2", target_bir_lowering=False, debug=False, num_devices=N_CORES
    )
    xh = nc.declare_dram_parameter("xh", [NCH, P, nD * CHK], bf16, isOutput=False)
    w1h = nc.declare_dram_parameter("w1h", [NSLOT, nFs, P, nD * P], bf16, isOutput=False)
    w2h = nc.declare_dram_parameter("w2h", [NSLOT, P, nFs * D], bf16, isOutput=False)
    b1h = nc.declare_dram_parameter("b1h", [P, NSLOT * nFs], f32, isOutput=False)
    yh = nc.declare_dram_parameter("yh", [NCH, P, nD * CHK], bf16, isOutput=True)

    XPRE = 8  # x chunks prefetched ahead of compute

    with tile.TileContext(nc) as tc:
        with (
            tc.tile_pool(name="const", bufs=1) as const_pool,
            tc.tile_pool(name="xpool", bufs=XPRE + 2) as xpool,
            tc.tile_pool(name="hpool", bufs=2) as hpool,
            tc.tile_pool(name="psum1", bufs=4, space="PSUM") as psum1,
            tc.tile_pool(name="psum2", bufs=3, space="PSUM") as psum2,
            tc.tile_pool(name="outp", bufs=3) as outp,
        ):
            # b1 first (tiny; the gelu activations carry it as a pointer
            # operand with only one sync-wait slot, so pre-touch it on the
            # scalar engine right away).
            b1_sb = const_pool.tile([P, NSLOT * nFs], f32)
            nc.sync.dma_start(b1_sb[:], b1h[:, :])
            scratch = const_pool.tile([P, 1], f32)
            nc.scalar.copy(scratch[:], b1_sb[:, 0:1])

            # Input DMAs interleaved so chunk-0 compute gates on only
            # ~1.3 MB (x0 + first half of slot0 w1) while later weights
            # stream behind the first chunks' compute.
            x_sb = {}

            def issue_x(ci):
                xt = xpool.tile([P, nD, CHK], bf16, tag="x")
                nc.sync.dma_start(xt[:], xh[ci].rearrange("p (d t) -> p d t", d=nD))
                x_sb[ci] = xt

            w1_sb = {}
            w2_sb = {}

            def issue_w1(s, fis):
                for fi in fis:
                    t = const_pool.tile([P, nD * P], bf16, tag=f"w1_{s}_{fi}")
                    nc.sync.dma_start(t[:], w1h[s, fi])
                    w1_sb[(s, fi)] = t

            def issue_w2(s):
                t = const_pool.tile([P, nFs, D], bf16, tag=f"w2_{s}")
                nc.sync.dma_start(t[:], w2h[s].rearrange("p (f d) -> p f d", f=nFs))
                w2_sb[s] = t

            slot_first = []
            for s, _ in sched:
                if s not in slot_first:
                    slot_first.append(s)
            s0, s1, s2, s3 = slot_first
            # First-chunk compute gates on b1 + w1[s0,fi0] + x0 (~1 MB);
            # everything later streams behind compute.
            issue_w1(s0, [0])
            issue_x(0)
            issue_w1(s0, [1, 2, 3, 4, 5])
            issue_x(1)
            issue_w2(s0)
            issue_x(2)
            issue_w1(s1, [0, 1, 2])
            issue_x(3)
            issue_w1(s1, [3, 4, 5])
            issue_w2(s1)
            issue_x(4)
            issue_w1(s2, [0, 1, 2])
            issue_x(5)
            issue_w1(s2, [3, 4, 5])
            issue_w2(s2)
            issue_x(6)
            issue_w1(s3, [0, 1, 2])
            issue_x(7)
            issue_w1(s3, [3, 4, 5])
            issue_w2(s3)

            def w1_tile(s, fi, d):
                return w1_sb[(s, fi)][:, d * P : (d + 1) * P]

            def w2_tile(s, fi, do):
                return w2_sb[s][:, fi, do * P : (do + 1) * P]

            # Dummy matmuls on a zeroed tile: keeps the PE active through
            # the HAM ramp (~3.3 us at half clock) while the first input
            # DMAs stream in, so real matmuls start at 2.4 GHz.
            warm_src = const_pool.tile([P, P], bf16)
            nc.any.memset(warm_src[:], 0.0)
            for _w in range(18):
                pw = psum1.tile([P, CHK], f32, tag="ph", name="pw")
                for k in range(4):
                    nc.tensor.matmul(
                        pw[:, :64],
                        lhsT=warm_src[:],
                        rhs=warm_src[:, :64],
                        start=(k == 0),
                        stop=(k == 3),
                    )

            h_of = {}

            def g1(ci):
                # h[f, tok] = gelu(sum_d w1[d, f] * x[d, tok] + b1[f])
                s, cs = sched[ci]
                h = hpool.tile([P, nFs, CHK], bf16, tag="h")
                for fi in range(nFs):
                    ph = psum1.tile([P, CHK], f32, tag="ph")
                    for d in range(nD):
                        nc.tensor.matmul(
                            ph[:, :cs],
                            lhsT=w1_tile(s, fi, d),
                            rhs=x_sb[ci][:, d, :cs],
                            start=(d == 0),
                            stop=(d == nD - 1),
                        )
                    nc.scalar.activation(
                        h[:, fi, :cs],
                        ph[:, :cs],
                        mybir.ActivationFunctionType.Gelu,
                        bias=b1_sb[:, s * nFs + fi : s * nFs + fi + 1],
                    )
                h_of[ci] = h

            def g2(ci):
                # y[dout, tok] = sum_f w2[f, dout] * h[f, tok]
                # do-major: each dout's psum completes early so its
                # copy-back overlaps the next dout's matmuls. The last
                # chunk stores per-dout (cs-wide) so the final DMA is
                # tiny and the kernel tail stays short.
                s, cs = sched[ci]
                last = ci == NCH - 1
                h = h_of.pop(ci)
                ot = outp.tile([P, nD, CHK], bf16, tag="ot")
                yh_r = yh[ci].rearrange("p (d t) -> p d t", d=nD)
                for do in range(nD):
                    py = psum2.tile([P, CHK], f32, tag="py")
                    for fi in range(nFs):
                        nc.tensor.matmul(
                            py[:, :cs],
                            lhsT=w2_tile(s, fi, do),
                            rhs=h[:, fi, :cs],
                            start=(fi == 0),
                            stop=(fi == nFs - 1),
                        )
                    nc.vector.tensor_copy(ot[:, do, :cs], py[:, :cs])
                    if last:
                        nc.sync.dma_start(
                            yh_r[:, do, :cs], ot[:, do, :cs]
                        )
                if not last:
                    nc.sync.dma_start(yh_r, ot[:])

            # Software pipeline: g1(c+1) issues between g1(c) and g2(c)
            # so every gelu has a full chunk of PE time to land before
            # its h is consumed, and the PE never waits on the scalar
            # engine at chunk boundaries.
            g1(0)
            for ci in range(1, NCH):
                if ci + XPRE - 1 < NCH:
                    issue_x(ci + XPRE - 1)
                g1(ci)
                g2(ci - 1)
            g2(NCH - 1)
    nc.compile()
    return nc


def _route(xf, router_w, router_b):
    """Top-2 routing, numpy fp32. Returns (idx1, idx2, g1, g2)."""
    logits = xf @ router_w + router_b
    m = logits.max(axis=-1, keepdims=True)
    p = np.exp(logits - m, dtype=np.float32)
    p /= p.sum(axis=-1, keepdims=True)
    # top-2 indices, ties -> lower index first (matches jax.lax.top_k)
    part = np.argpartition(-p, 1, axis=-1)[:, :2]
    pv = np.take_along_axis(p, part, axis=-1)
    swap = (pv[:, 1] > pv[:, 0]) | ((pv[:, 1] == pv[:, 0]) & (part[:, 1] < part[:, 0]))
    i1 = np.where(swap, part[:, 1], part[:, 0])
    i2 = np.where(swap, part[:, 0], part[:, 1])
    p1 = np.take_along_axis(p, i1[:, None], axis=-1)[:, 0]
    p2 = np.take_along_axis(p, i2[:, None], axis=-1)[:, 0]
    s = p1 + p2
    return i1, i2, p1 / s, p2 / s


def _ceil8(n):
    return -(-n // 8) * 8


def kernel(x, router_w, router_b, w1, b1, w2, b2):
    global last_results
    x = np.asarray(x, dtype=np.float32)
    router_w = np.asarray(router_w, dtype=np.float32)
    router_b = np.asarray(router_b, dtype=np.float32)
    w1 = np.asarray(w1, dtype=np.float32)
    b1 = np.asarray(b1, dtype=np.float32)
    w2 = np.asarray(w2, dtype=np.float32)
    b2 = np.asarray(b2, dtype=np.float32)

    B, S, _ = x.shape
    T = B * S
    xf = x.reshape(T, D)

    i1, i2, g1_, g2_ = _route(xf, router_w, router_b)

    tok_lists = []
    gate_lists = []
    for e in range(E):
        m1 = i1 == e
        m2 = i2 == e
        toks = np.nonzero(m1 | m2)[0]
        gates = np.where(m1[toks], g1_[toks], g2_[toks]).astype(np.float32)
        tok_lists.append(toks)
        gate_lists.append(gates)
    counts = np.array([len(t) for t in tok_lists])

    # Split experts into 2 quads of 4 minimizing the shared (slot-wise max)
    # schedule size. Each quad's experts sorted desc onto slots.
    best = None
    for combo in combinations(range(E), NSLOT):
        if 0 not in combo:
            continue
        qa = sorted(combo, key=lambda e: -counts[e])
        qb = sorted(set(range(E)) - set(combo), key=lambda e: -counts[e])
        sizes = [
            _ceil8(max(counts[qa[k]], counts[qb[k]])) for k in range(NSLOT)
        ]
        tot = sum(sizes)
        if best is None or tot < best[0]:
            best = (tot, qa, qb, sizes)
    _, quad_a, quad_b, slot_sizes = best
    quads = [quad_a, quad_b]

    # Process slots so the final chunk of the kernel is the smallest one.
    slot_chunks = [_chunks_of(slot_sizes[s], CHK) for s in range(NSLOT)]
    slot_order = sorted(range(NSLOT), key=lambda s: -slot_chunks[s][-1][1])
    # Flattened schedule: list of (slot, cs) plus per-chunk column ranges.
    sched = []
    chunk_cols = []  # (slot, t0_in_slot, cs)
    for s in slot_order:
        for t0, cs in slot_chunks[s]:
            sched.append((s, cs))
            chunk_cols.append((s, t0, cs))
    NCH = len(sched)

    nc = _build(sched)

    xf_b = xf.astype(ml_dtypes.bfloat16)
    w1_b = w1.astype(ml_dtypes.bfloat16)
    w2_b = w2.astype(ml_dtypes.bfloat16)

    in_maps = []
    for q in range(2):
        # Per-slot xT [D, size] then cut into pre-tiled chunks.
        xh = np.zeros((NCH, P, nD * CHK), dtype=ml_dtypes.bfloat16)
        xslot = {}
        for s in range(NSLOT):
            e = quads[q][s]
            toks = tok_lists[e]
            xs = np.zeros((D, slot_sizes[s]), dtype=ml_dtypes.bfloat16)
            xs[:, : len(toks)] = xf_b[toks].T
            xslot[s] = xs
        for ci, (s, t0, cs) in enumerate(chunk_cols):
            blk = np.zeros((nD, P, CHK), dtype=ml_dtypes.bfloat16)
            blk[:, :, :cs] = xslot[s][:, t0 : t0 + cs].reshape(nD, P, cs)
            xh[ci] = blk.transpose(1, 0, 2).reshape(P, nD * CHK)
        for fs in range(NSLOT):
            lo = fs * FS
            w1h = np.empty((NSLOT, nFs, P, nD * P), dtype=ml_dtypes.bfloat16)
            w2h = np.empty((NSLOT, P, nFs * D), dtype=ml_dtypes.bfloat16)
            b1h = np.empty((P, NSLOT * nFs), dtype=np.float32)
            for s in range(NSLOT):
                e = quads[q][s]
                # w1 slice [D, FS]: w1h[s, f, p, d*P + c]
                #   = w1slice[d*P + p, f*P + c]
                w1s = w1_b[e][:, lo : lo + FS]
                w1h[s] = w1s.reshape(nD, P, nFs, P).transpose(2, 1, 0, 3).reshape(
                    nFs, P, nD * P
                )
                # w2 slice [FS, D]: w2h[s, p, f*D + d] = w2slice[f*P+p, d]
                w2s = w2_b[e][lo : lo + FS]
                w2h[s] = w2s.reshape(nFs, P, D).transpose(1, 0, 2).reshape(P, nFs * D)
                b1h[:, s * nFs : (s + 1) * nFs] = (
                    b1[e][lo : lo + FS].reshape(nFs, P).T
                )
            in_maps.append({"xh": xh, "w1h": w1h, "w2h": w2h, "b1h": b1h})

    trace = bool(int(os.environ.get("KERNEL_TRACE", "0")))
    last_results = run_bass_kernel_spmd(
        nc, in_maps, core_ids=list(range(N_CORES)), trace=trace
    )

    out = np.zeros((T, D), dtype=np.float32)
    for q in range(2):
        ysum = np.zeros((NCH, P, nD * CHK), dtype=np.float32)
        for fs in range(NSLOT):
            ysum += last_results.results[q * NSLOT + fs]["yh"].astype(np.float32)
        # yh[c, p, o*CHK + t] -> per-chunk [D, cs] blocks
        ysum = ysum.reshape(NCH, P, nD, CHK)
        yslot = {s: np.empty((D, slot_sizes[s]), dtype=np.float32) for s in range(NSLOT)}
        for ci, (s, t0, cs) in enumerate(chunk_cols):
            yslot[s][:, t0 : t0 + cs] = (
                ysum[ci, :, :, :cs].transpose(1, 0, 2).reshape(D, cs)
            )
        for s in range(NSLOT):
            e = quads[q][s]
            toks = tok_lists[e]
            ye = yslot[s][:, : len(toks)].T
            out[toks] += gate_lists[e][:, None] * (ye + b2[e][None, :])
    return out.reshape(B, S, D)
